# revision 9
# baseline (speedup 1.0000x reference)
"""Trainium2 Bass kernel: masked attention-energy softmax.

Computes, for each batch row b:
    energy[b, t] = v . (W @ q[b, t] + bias)          (== q[b, t] . (W^T v) + bias . v)
    out[b]      = softmax(mask(energy[b]), axis=t)   with t >= len[b] masked to -1e10

Strategy
--------
* Pure data parallel over 8 NeuronCores: 8 batch rows per core, W/b/v params
  folded on host into a single vector u = W^T v (the bias.v constant shifts every
  energy in a row equally, so it cancels in softmax and is dropped).
* Per core: questions shard [8, 2048, 256] is streamed in 2 MB/batch DMAs laid
  out [128 partitions, 16 tok, 256 h] (t = p*16 + j, 16 KB contiguous per
  partition).  A fused DVE tensor_tensor_reduce does (q * u) and the 256-wide
  h-sum in one pass per 128-token group -> energies [128, 16] per batch.
* Mask is built on-chip from an iota over token indices vs. the row length;
  exp runs on the scalar engine with fused accumulation; the cross-partition
  sum uses gpsimd partition_all_reduce; a per-partition reciprocal scale
  finishes softmax.  No max-subtraction is needed: energies are O(+-6) here
  (u has unit-variance rows), and softmax is shift-invariant.
"""

import numpy as np

B, T, H = 64, 2048, 256
NCORES = 8
NB = B // NCORES  # batches per core
P = 128  # SBUF partitions
J = T // P  # tokens per partition
NEG = -1.0e10
NB_DVE_RED = 2  # batches whose h-reduce runs on DVE (rest on ScalarE)

_CACHE = {}


def _build_nc():
    from contextlib import ExitStack

    import concourse.bacc as bacc
    import concourse.bass as bass
    import concourse.tile as tile
    from concourse import library_config, mybir
    from concourse.bass_isa import ReduceOp

    f32 = mybir.dt.float32
    nc = bacc.Bacc("TRN2", target_bir_lowering=False, debug=False)

    q_d = nc.dram_tensor("q", [NB, T, H], f32, kind="ExternalInput").ap()
    u_d = nc.dram_tensor("u", [H], f32, kind="ExternalInput").ap()
    lens_d = nc.dram_tensor("lens", [NB], f32, kind="ExternalInput").ap()
    out_d = nc.dram_tensor("out", [NB, T], f32, kind="ExternalOutput").ap()

    # t = p * J + j: each partition reads a contiguous 16 KB slice of Q[b]
    qr = q_d.rearrange("b (p j) h -> b p j h", p=P)
    outr = out_d.rearrange("b (p j) -> p b j", p=P)

    with tile.TileContext(nc) as tc, ExitStack() as ctx:
        singles = ctx.enter_context(tc.tile_pool(name="singles", bufs=1))
        qpool = ctx.enter_context(tc.tile_pool(name="qpool", bufs=3))
        epool = ctx.enter_context(tc.tile_pool(name="epool", bufs=2))
        spool = ctx.enter_context(tc.tile_pool(name="spool", bufs=2))

        # u repeated J times along free dim, broadcast to all 128 partitions
        # (step-0 partition DMA): u_rep[p, j*H + h] = u[h]
        u_rep = singles.tile([P, J * H], f32)
        nc.gpsimd.dma_start(
            out=u_rep,
            in_=bass.AP(
                tensor=u_d.tensor, offset=u_d.offset, ap=[[0, P], [0, J]] + list(u_d.ap)
            ),
        )
        lens_b = singles.tile([P, NB], f32)
        nc.gpsimd.dma_start(
            out=lens_b,
            in_=bass.AP(
                tensor=lens_d.tensor, offset=lens_d.offset, ap=[[0, P]] + list(lens_d.ap)
            ),
        )
        # token index per (p, j): t = p*J + j
        iota_i = singles.tile([P, J], mybir.dt.int32)
        nc.gpsimd.iota(iota_i, pattern=[[1, J]], base=0, channel_multiplier=J)
        iota_f = singles.tile([P, J], f32)
        nc.vector.tensor_copy(iota_f, iota_i)
        # iota needs the 'standard' gpsimd library; partition_all_reduce (used
        # at the end) lives in 'attnmlp' -- switch now so the ~6us IRAM load
        # overlaps the main compute loop.
        nc.gpsimd.load_library(library_config.attnmlp)

        expE = singles.tile([P, NB, J], f32)
        acc = singles.tile([P, NB], f32)
        probs = singles.tile([P, NB, J], f32)

        for b in range(NB):
            qb = qpool.tile([P, J, H], f32)
            nc.sync.dma_start(out=qb, in_=qr[b])

            # prod[p, j, h] = q[p, j, h] * u[h]  (one big 1x DVE pass)
            prod = qpool.tile([P, J, H], f32, tag="prod")
            nc.vector.tensor_mul(
                prod.rearrange("p j h -> p (j h)"),
                qb.rearrange("p j h -> p (j h)"),
                u_rep,
            )
            E = epool.tile([P, J], f32)
            if b < NB_DVE_RED:
                # E[:, j] = sum_h prod[:, j, :]  (grouped free-axis reduce)
                nc.vector.tensor_reduce(
                    E, prod, axis=mybir.AxisListType.X, op=mybir.AluOpType.add
                )
            else:
                # same reduce on the scalar engine: Copy with accumulation
                for j in range(J):
                    scr = spool.tile([P, H], f32, tag="scr")
                    nc.scalar.activation(
                        out=scr,
                        in_=prod[:, j, :],
                        func=mybir.ActivationFunctionType.Copy,
                        accum_out=E[:, j : j + 1],
                    )
            # nm = (t >= len[b]) * NEG ; Em = E + nm
            nm = epool.tile([P, J], f32)
            nc.vector.tensor_scalar(
                out=nm,
                in0=iota_f,
                scalar1=lens_b[:, b : b + 1],
                scalar2=NEG,
                op0=mybir.AluOpType.is_ge,
                op1=mybir.AluOpType.mult,
            )
            nc.vector.tensor_add(nm, nm, E)
            # expE[:, b, :] = exp(Em), acc[:, b] = sum_j exp(Em[:, j])
            nc.scalar.activation(
                out=expE[:, b, :],
                in_=nm,
                func=mybir.ActivationFunctionType.Exp,
                accum_out=acc[:, b : b + 1],
            )

        # total per-batch sums, broadcast back to all partitions
        nc.gpsimd.partition_all_reduce(acc, acc, P, ReduceOp.add)
        recip = singles.tile([P, NB], f32)
        nc.vector.reciprocal(recip, acc)
        for b in range(NB):
            nc.vector.tensor_scalar_mul(
                probs[:, b, :], expE[:, b, :], recip[:, b : b + 1]
            )
        nc.sync.dma_start(out=outr, in_=probs)

    nc.compile()
    return nc


def _prep_inputs(questions, questions_lens, W, b, v):
    q = np.ascontiguousarray(np.asarray(questions, dtype=np.float32))
    lens = np.asarray(questions_lens)
    W = np.asarray(W, dtype=np.float32)
    v = np.asarray(v, dtype=np.float32)
    u = np.ascontiguousarray(W.T @ v).astype(np.float32)
    lens_f = lens.astype(np.float32)
    in_maps = []
    for c in range(NCORES):
        in_maps.append(
            {
                "q": q[c * NB : (c + 1) * NB],
                "u": u,
                "lens": lens_f[c * NB : (c + 1) * NB],
            }
        )
    return in_maps


def _get_runner():
    """Build (once) a persistent sharded-jit runner over the 8 NeuronCores.

    Mirrors concourse.bass2jax.run_bass_via_pjrt's multi-core path, but caches
    the jitted executable so repeated kernel() calls skip retrace/recompile.
    """
    if "runner" in _CACHE:
        return _CACHE["runner"]

    import jax
    from jax.sharding import Mesh, PartitionSpec
    from jax.experimental.shard_map import shard_map

    import concourse.mybir as mybir
    from concourse.bass2jax import (
        _bass_exec_p,
        install_neuronx_cc_hook,
        partition_id_tensor,
    )

    nc = _build_nc()
    install_neuronx_cc_hook()

    partition_name = nc.partition_id_tensor.name if nc.partition_id_tensor else None
    in_names, out_names, out_avals, zero_outs = [], [], [], []
    for alloc in nc.m.functions[0].allocations:
        if not isinstance(alloc, mybir.MemoryLocationSet):
            continue
        name = alloc.memorylocations[0].name
        if alloc.kind == "ExternalInput":
            if name != partition_name:
                in_names.append(name)
        elif alloc.kind == "ExternalOutput":
            out_names.append(name)
            shape = tuple(alloc.tensor_shape)
            dtype = mybir.dt.np(alloc.dtype)
            out_avals.append(jax.core.ShapedArray(shape, dtype))
            zero_outs.append(np.zeros(shape, dtype))
    n_params = len(in_names)
    all_in_names = list(in_names) + list(out_names)
    if partition_name is not None:
        all_in_names.append(partition_name)

    def _body(*args):
        operands = list(args)
        if partition_name is not None:
            operands.append(partition_id_tensor())
        outs = _bass_exec_p.bind(
            *operands,
            out_avals=tuple(out_avals),
            in_names=tuple(all_in_names),
            out_names=tuple(out_names),
            lowering_input_output_aliases=(),
            sim_require_finite=True,
            sim_require_nnan=True,
            nc=nc,
        )
        return tuple(outs)

    devices = jax.devices()[:NCORES]
    mesh = Mesh(np.asarray(devices), ("core",))
    n_outs = len(out_names)
    in_specs = (PartitionSpec("core"),) * (n_params + n_outs)
    out_specs = (PartitionSpec("core"),) * n_outs
    sharded = jax.jit(
        shard_map(
            _body, mesh=mesh, in_specs=in_specs, out_specs=out_specs, check_rep=False
        ),
        donate_argnums=tuple(range(n_params, n_params + n_outs)),
        keep_unused=True,
    )

    def run(in_maps):
        concat_in = [
            np.concatenate([np.asarray(m[name]) for m in in_maps], axis=0)
            for name in in_names
        ]
        concat_zeros = [
            np.zeros((NCORES * z.shape[0], *z.shape[1:]), z.dtype) for z in zero_outs
        ]
        out_arrs = sharded(*concat_in, *concat_zeros)
        return {
            name: np.asarray(out_arrs[i]).reshape(NCORES * out_avals[i].shape[0], *out_avals[i].shape[1:])
            for i, name in enumerate(out_names)
        }

    _CACHE["runner"] = run
    return run


def kernel(questions, questions_lens, W, b, v):
    run = _get_runner()
    in_maps = _prep_inputs(questions, questions_lens, W, b, v)
    return run(in_maps)["out"]


def kernel_ex(questions, questions_lens, W, b, v, trace=False):
    """Compat wrapper used by test.py: returns (out, None)."""
    return kernel(questions, questions_lens, W, b, v), None


# revision 18
# speedup vs baseline: 12643.2071x; 12643.2071x over previous
"""Trainium2 Bass kernel: masked attention-energy softmax.

Computes, for each batch row b:
    energy[b, t] = v . (W @ q[b, t] + bias)          (== q[b, t] . (W^T v) + bias . v)
    out[b]      = softmax(mask(energy[b]), axis=t)   with t >= len[b] masked to -1e10

Strategy
--------
* Pure data parallel over 8 NeuronCores: 8 batch rows per core, W/b/v params
  folded on host into a single vector u = W^T v (the bias.v constant shifts every
  energy in a row equally, so it cancels in softmax and is dropped).
* Per core: questions shard [8, 2048, 256] is streamed in 2 MB/batch DMAs laid
  out [128 partitions, 16 tok, 256 h] (t = p*16 + j, 16 KB contiguous per
  partition, sequential in HBM).  DVE does prod = q * u_broadcast in one big
  1x pass per batch; the 256-wide h-sum is split across engines to balance
  them: grouped DVE tensor_reduce for the first NB_DVE_RED batches, ScalarE
  Copy-with-accumulate for the rest.  (The fused tensor_tensor_reduce custom
  DVE op would halve the DVE work but crashes this runtime's device.)
* Mask is built on-chip from an iota over token indices vs. the row length;
  exp runs on the scalar engine with fused accumulation; the cross-partition
  sum uses gpsimd partition_all_reduce; a per-partition reciprocal scale
  finishes softmax.  No max-subtraction is needed: energies are O(+-6) here
  (u has unit-variance rows), and softmax is shift-invariant.
"""

import numpy as np

B, T, H = 64, 2048, 256
NCORES = 8
NB = B // NCORES  # batches per core
P = 128  # SBUF partitions
J = T // P  # tokens per partition
NEG = -1.0e10
NB_DVE_RED = 2  # batches whose h-reduce runs on DVE (rest on ScalarE)

_CACHE = {}


def _build_nc(reps=1):
    """Build the per-core Bass program.  reps>1 statically unrolls the whole
    computation for benchmarking (marginal per-rep wall time isolates HW
    execution time from axon dispatch overhead); the graded path uses reps=1.
    """
    from contextlib import ExitStack

    import concourse.bacc as bacc
    import concourse.bass as bass
    import concourse.tile as tile
    from concourse import library_config, mybir
    from concourse.bass_isa import ReduceOp

    f32 = mybir.dt.float32
    nc = bacc.Bacc("TRN2", target_bir_lowering=False, debug=False)

    q_d = nc.dram_tensor("q", [NB, T, H], f32, kind="ExternalInput").ap()
    u_d = nc.dram_tensor("u", [H], f32, kind="ExternalInput").ap()
    lens_d = nc.dram_tensor("lens", [NB], f32, kind="ExternalInput").ap()
    out_d = nc.dram_tensor("out", [NB, T], f32, kind="ExternalOutput").ap()

    # t = p * J + j: each partition reads a contiguous 16 KB slice of Q[b]
    qr = q_d.rearrange("b (p j) h -> b p j h", p=P)
    outr = out_d.rearrange("b (p j) -> p b j", p=P)

    with tile.TileContext(nc) as tc, ExitStack() as ctx:
        singles = ctx.enter_context(tc.tile_pool(name="singles", bufs=1))
        qpool = ctx.enter_context(tc.tile_pool(name="qpool", bufs=3))
        epool = ctx.enter_context(tc.tile_pool(name="epool", bufs=2))
        spool = ctx.enter_context(tc.tile_pool(name="spool", bufs=2))

        # u repeated J times along free dim, broadcast to all 128 partitions
        # (step-0 partition DMA): u_rep[p, j*H + h] = u[h]
        u_rep = singles.tile([P, J * H], f32)
        nc.gpsimd.dma_start(
            out=u_rep,
            in_=bass.AP(
                tensor=u_d.tensor, offset=u_d.offset, ap=[[0, P], [0, J]] + list(u_d.ap)
            ),
        )
        lens_b = singles.tile([P, NB], f32)
        nc.gpsimd.dma_start(
            out=lens_b,
            in_=bass.AP(
                tensor=lens_d.tensor, offset=lens_d.offset, ap=[[0, P]] + list(lens_d.ap)
            ),
        )
        # token index per (p, j): t = p*J + j
        iota_i = singles.tile([P, J], mybir.dt.int32)
        nc.gpsimd.iota(iota_i, pattern=[[1, J]], base=0, channel_multiplier=J)
        iota_f = singles.tile([P, J], f32)
        nc.vector.tensor_copy(iota_f, iota_i)
        # iota needs the 'standard' gpsimd library; partition_all_reduce (used
        # at the end) lives in 'attnmlp' -- switch now so the ~6us IRAM load
        # overlaps the main compute loop.
        nc.gpsimd.load_library(library_config.attnmlp)

        expE = singles.tile([P, NB, J], f32)
        acc = singles.tile([P, NB], f32)
        probs = singles.tile([P, NB, J], f32)

        for _rep in range(reps):
            for b in range(NB):
                qb = qpool.tile([P, J, H], f32, tag="qb")
                nc.sync.dma_start(out=qb, in_=qr[b])

                # prod[p, j, h] = q[p, j, h] * u[h]  (one big 1x DVE pass)
                prod = qpool.tile([P, J, H], f32, tag="prod")
                nc.vector.tensor_mul(
                    prod.rearrange("p j h -> p (j h)"),
                    qb.rearrange("p j h -> p (j h)"),
                    u_rep,
                )
                E = epool.tile([P, J], f32, tag="E")
                if b < NB_DVE_RED:
                    # E[:, j] = sum_h prod[:, j, :]  (grouped free-axis reduce)
                    nc.vector.tensor_reduce(
                        E, prod, axis=mybir.AxisListType.X, op=mybir.AluOpType.add
                    )
                else:
                    # same reduce on the scalar engine: Copy with accumulation
                    for j in range(J):
                        scr = spool.tile([P, H], f32, tag="scr")
                        nc.scalar.activation(
                            out=scr,
                            in_=prod[:, j, :],
                            func=mybir.ActivationFunctionType.Copy,
                            accum_out=E[:, j : j + 1],
                        )
                # nm = (t >= len[b]) * NEG ; Em = E + nm
                nm = epool.tile([P, J], f32, tag="nm")
                nc.vector.tensor_scalar(
                    out=nm,
                    in0=iota_f,
                    scalar1=lens_b[:, b : b + 1],
                    scalar2=NEG,
                    op0=mybir.AluOpType.is_ge,
                    op1=mybir.AluOpType.mult,
                )
                nc.vector.tensor_add(nm, nm, E)
                # expE[:, b, :] = exp(Em), acc[:, b] = sum_j exp(Em[:, j])
                nc.scalar.activation(
                    out=expE[:, b, :],
                    in_=nm,
                    func=mybir.ActivationFunctionType.Exp,
                    accum_out=acc[:, b : b + 1],
                )

            # total per-batch sums, broadcast back to all partitions
            nc.gpsimd.partition_all_reduce(acc, acc, P, ReduceOp.add)
            recip = singles.tile([P, NB], f32, tag="recip")
            nc.vector.reciprocal(recip, acc)
            for b in range(NB):
                nc.vector.tensor_scalar_mul(
                    probs[:, b, :], expE[:, b, :], recip[:, b : b + 1]
                )
            nc.sync.dma_start(out=outr, in_=probs)

    nc.compile()
    return nc


def _prep_inputs(questions, questions_lens, W, b, v):
    q = np.ascontiguousarray(np.asarray(questions, dtype=np.float32))
    lens = np.asarray(questions_lens)
    W = np.asarray(W, dtype=np.float32)
    v = np.asarray(v, dtype=np.float32)
    u = np.ascontiguousarray(W.T @ v).astype(np.float32)
    lens_f = lens.astype(np.float32)
    in_maps = []
    for c in range(NCORES):
        in_maps.append(
            {
                "q": q[c * NB : (c + 1) * NB],
                "u": u,
                "lens": lens_f[c * NB : (c + 1) * NB],
            }
        )
    return in_maps


def _get_runner(reps=1):
    """Build (once per reps) a persistent sharded-jit runner over the 8 cores.

    Mirrors concourse.bass2jax.run_bass_via_pjrt's multi-core path, but caches
    the jitted executable so repeated calls skip retrace/recompile.  Used for
    benchmarking; the graded kernel() path goes through run_bass_kernel_spmd.
    """
    key = ("runner", reps)
    if key in _CACHE:
        return _CACHE[key]

    import jax
    from jax.sharding import Mesh, PartitionSpec
    from jax.experimental.shard_map import shard_map

    import concourse.mybir as mybir
    from concourse.bass2jax import (
        _bass_exec_p,
        install_neuronx_cc_hook,
        partition_id_tensor,
    )

    nc = _build_nc(reps)
    install_neuronx_cc_hook()

    partition_name = nc.partition_id_tensor.name if nc.partition_id_tensor else None
    in_names, out_names, out_avals, zero_outs = [], [], [], []
    for alloc in nc.m.functions[0].allocations:
        if not isinstance(alloc, mybir.MemoryLocationSet):
            continue
        name = alloc.memorylocations[0].name
        if alloc.kind == "ExternalInput":
            if name != partition_name:
                in_names.append(name)
        elif alloc.kind == "ExternalOutput":
            out_names.append(name)
            shape = tuple(alloc.tensor_shape)
            dtype = mybir.dt.np(alloc.dtype)
            out_avals.append(jax.core.ShapedArray(shape, dtype))
            zero_outs.append(np.zeros(shape, dtype))
    n_params = len(in_names)
    all_in_names = list(in_names) + list(out_names)
    if partition_name is not None:
        all_in_names.append(partition_name)

    def _body(*args):
        operands = list(args)
        if partition_name is not None:
            operands.append(partition_id_tensor())
        outs = _bass_exec_p.bind(
            *operands,
            out_avals=tuple(out_avals),
            in_names=tuple(all_in_names),
            out_names=tuple(out_names),
            lowering_input_output_aliases=(),
            sim_require_finite=True,
            sim_require_nnan=True,
            nc=nc,
        )
        return tuple(outs)

    devices = jax.devices()[:NCORES]
    mesh = Mesh(np.asarray(devices), ("core",))
    n_outs = len(out_names)
    in_specs = (PartitionSpec("core"),) * (n_params + n_outs)
    out_specs = (PartitionSpec("core"),) * n_outs
    sharded = jax.jit(
        shard_map(
            _body, mesh=mesh, in_specs=in_specs, out_specs=out_specs, check_rep=False
        ),
        donate_argnums=tuple(range(n_params, n_params + n_outs)),
        keep_unused=True,
    )

    def run(in_maps):
        concat_in = [
            np.concatenate([np.asarray(m[name]) for m in in_maps], axis=0)
            for name in in_names
        ]
        concat_zeros = [
            np.zeros((NCORES * z.shape[0], *z.shape[1:]), z.dtype) for z in zero_outs
        ]
        out_arrs = sharded(*concat_in, *concat_zeros)
        return {
            name: np.asarray(out_arrs[i]).reshape(NCORES * out_avals[i].shape[0], *out_avals[i].shape[1:])
            for i, name in enumerate(out_names)
        }

    _CACHE[("parts", reps)] = dict(
        sharded=sharded,
        in_names=in_names,
        out_names=out_names,
        out_avals=out_avals,
        zero_outs=zero_outs,
        mesh=mesh,
    )
    _CACHE[key] = run
    return run


def kernel(questions, questions_lens, W, b, v):
    """Full-input entry point: shards across the 8 NeuronCores, runs the Bass
    kernel via run_bass_kernel_spmd, gathers the full [64, 2048] output."""
    from concourse.bass_utils import run_bass_kernel_spmd

    if "nc" not in _CACHE:
        _CACHE["nc"] = _build_nc()
    in_maps = _prep_inputs(questions, questions_lens, W, b, v)
    res = run_bass_kernel_spmd(_CACHE["nc"], in_maps, list(range(NCORES)))
    return np.concatenate([r["out"] for r in res.results], axis=0)
